# revision 46
# baseline (speedup 1.0000x reference)
"""Trainium2 Bass kernel for nn_ARAttention, v5.

Structure (per core, R=4 batch rows; data-parallel over 8 cores):
- argmax of prev_attention via 2-level max (per-partition max8/maxidx, PE
  transpose, free-major reduce with tie-safe min-index trick)
- band window [s0, s0+31] gathered from enc via one 128-descriptor SWDGE
  indirect DMA; conv windows via 4 register-offset HWDGE DMAs
- e = softsign(W_enc x + b) + conv + v  (v = prenet terms, fp8 weights
  with x64 scaling; biases are all zero in this model — bp1 is folded into
  wp1b via a constant-1 input channel, the rest are assert-checked zero)
- tanh -> W_proj matmul with -60 band-mask bias rows -> exp -> per-row
  normalize -> one 4-descriptor SWDGE indirect scatter into the zeroed
  output.

Engine discipline: DVE runs ONLY the argmax->soffs critical chain plus the
post-PS softsign chain; band mask, v-combine, and B=PC+v run on Pool; PE
order is transposes -> prenet -> enc matmuls; Act does relus/tanh/exp and
big-image DMA issue. This keeps Tile's sem-coalescing from serializing the
critical chain behind slack work.
"""

import numpy as np

import concourse.bass as bass
import concourse.mybir as mybir

# -- walrus "too many sync waits" workaround (same as v1/v2) ----------------
import concourse.tile as tile
from concourse.vector_clock import VectorClock, ScopedClock


def _patched_drain_and_barrier(self, tick_clock, wait_clock):
    nc = self.nc
    gc = list(tick_clock.global_clock)
    for p, tick in enumerate(gc):
        if tick > 0:
            nop = nc.sync.nop(nofuse=True)
            partial = VectorClock([tick if i == p else 0 for i in range(len(gc))])
            wait_clock.add_sem_waits(nop.ins, ScopedClock({None: partial}))
    nc.sync.drain()
    nc.all_engine_barrier()
    assert self.sems is not None
    popped = nc._tile_sem_poison_stack.pop()
    assert popped is self._sem_poison
    nc.clear_and_free_semaphores(list(self.sems.allocated().values()))
    nc.all_engine_barrier()


tile.TileContext._drain_and_barrier = _patched_drain_and_barrier
# ---------------------------------------------------------------------------

from concourse.masks import make_identity

f32 = mybir.dt.float32
bf16 = mybir.dt.bfloat16
fp8 = mybir.dt.float8e4
i32 = mybir.dt.int32
u32 = mybir.dt.uint32
AF = mybir.ActivationFunctionType
ALU = mybir.AluOpType

N, T, ENC_H, ATT_H, DEC_H, OUTD, SPK = 32, 4096, 512, 256, 512, 80, 64
ATT_RANGE, KW = 10, 31
NCORES = 8
R = N // NCORES
W = 32
PAD = 15
TP = PAD + T + PAD + 2   # 4128
P = 128
FS = 64.0                # fp8 weight scaling

# f32 image columns
C_PA = 0          # [128,128] pa reshaped (r t)->(a b), b=128
C_LM1 = 128       # [1,4] row0: lengths-1
C_JI = 132       # [1,128] row0: -T*(c>>5)  (mask pos correction)
C_J32 = 260       # [128,1] per-partition: (p&31) + T*(p>>5)
C_CBF = 261       # [128,1] per-partition: 8191 - (p&31)*128
C_R4 = 262        # [1,4] row0: r*T
C_SP = 266        # [1,128] row0: (c&31) + T*(c>>5) (soffs spread add)
SMC = 394

# bf16 image columns
B_DST0 = 0        # [128,4] dec_in rows 0..127 (transposed)
B_DST1 = 4        # [17,4] dec_in rows 128..143 + ones row
B_SPK = 8         # [64,4]
B_SPD = 12        # [1,4]
B_WSPK = 16       # [64,256] (x64 NOT applied; accumulated with PK alone)
B_WSPD = 272      # [1,256] x64
B_WPROJ = 528     # [128,2]
B_CW = 530        # [31,256] conv weights transposed
SMB = 786

# fp8 weight image columns (all x64-scaled)
F_WP1 = 0         # [128,1024]
F_WP2 = 1024      # [128,4096]
F_WDEC = 5120     # [128,1024]
SMF = 6144


def _emit_core(nc, tc, ctx, x):
    cp = ctx.enter_context(tc.tile_pool(name="cp", bufs=1))
    wp = ctx.enter_context(tc.tile_pool(name="wp", bufs=1))
    pt = ctx.enter_context(tc.tile_pool(name="pt", bufs=2, space="PSUM"))
    pq = ctx.enter_context(tc.tile_pool(name="pq", bufs=2, space="PSUM"))
    pr = ctx.enter_context(tc.tile_pool(name="pr", bufs=1, space="PSUM"))
    pc = ctx.enter_context(tc.tile_pool(name="pc", bufs=1, space="PSUM"))

    ident = cp.tile([P, P], f32)
    make_identity(nc, ident[:])
    identb = cp.tile([P, P], bf16)
    nc.scalar.copy(identb[:], ident[:])
    ones1 = cp.tile([1, P], bf16)
    nc.gpsimd.memset(ones1[:], 1.0)
    Z = cp.tile([P, P], f32)
    nc.vector.memset(Z[:], 0.0)

    # ---- static input DMAs (issue order ~ need order) --------------------
    sm = wp.tile([P, SMC], f32)
    with tc.high_priority():
        nc.sync.dma_start(sm[:], x["img"][:])
    ia = wp.tile([P, SMB], bf16)
    nc.scalar.dma_start(ia[:], x["imga"][:])
    w8 = wp.tile([P, SMF], fp8)
    nc.scalar.dma_start(w8[:], x["w8"][:])
    wp1b = wp.tile([17, 2 * DEC_H], fp8)
    nc.sync.dma_start(wp1b[:], x["wp1b"][:])
    wenc = wp.tile([P, 4 * ATT_H], bf16)
    nc.sync.dma_start(wenc[:], x["wenc"][:])
    out_flat2 = x["out"][:].rearrange("a b -> (a b)").rearrange("(p f) -> p f", f=P)
    zero_dma = nc.sync.dma_start(out=out_flat2, in_=Z[:])

    pa128 = sm[:, C_PA : C_PA + P]

    # ================= prenet layer 1 on PE ===============================
    PH = pq.tile([P, 8 * R], f32, tag="sm")
    for m in range(8):
        nc.tensor.matmul(
            PH[:, m * R : (m + 1) * R],
            lhsT=w8[:, F_WP1 + m * P : F_WP1 + (m + 1) * P],
            rhs=ia[:, B_DST0 : B_DST0 + R], start=True, stop=False,
        )
        nc.tensor.matmul(
            PH[:, m * R : (m + 1) * R],
            lhsT=wp1b[0:17, m * P : (m + 1) * P],
            rhs=ia[0:17, B_DST1 : B_DST1 + R], start=False, stop=True,
        )
    HTrB = wp.tile([P, 8 * R], bf16)
    nc.scalar.activation(HTrB[:], PH[:], AF.Relu, scale=1.0 / FS)

    # ================= argmax level 1 (DVE) ===============================
    mx8 = cp.tile([P, 8], f32)
    nc.vector.max(out=mx8[:], in_=pa128)
    mi8 = cp.tile([P, 8], u32)
    nc.vector.max_index(out=mi8[:], in_max=mx8[:], in_values=pa128)
    # gidxp = (8191 - cb) - mi  (min-index tie-break via inverted score)
    gidxp = cp.tile([P, 1], f32)
    nc.vector.tensor_tensor(
        out=gidxp[:], in0=sm[:, C_CBF : C_CBF + 1], in1=mi8[:, 0:1], op=ALU.subtract
    )
    with tc.high_priority():
        vT = pt.tile([1, P], f32, tag="tp")
        nc.tensor.transpose(out=vT[:], in_=mx8[:, 0:1], identity=ident[:])
        gT = pt.tile([1, P], f32, tag="tp")
        nc.tensor.transpose(out=gT[:], in_=gidxp[:], identity=ident[:])

    # ================= argmax level 2 (free-major [1,128]) ================
    M = cp.tile([1, R], f32)
    nc.vector.reduce_max(
        out=M[:], in_=vT[0:1, :].rearrange("p (r q) -> p r q", q=32),
        axis=mybir.AxisListType.X,
    )
    eq = cp.tile([1, P], f32)
    nc.vector.tensor_tensor(
        out=eq[:].rearrange("p (r q) -> p r q", q=32),
        in0=vT[0:1, :].rearrange("p (r q) -> p r q", q=32),
        in1=M[:].to_broadcast([1, R, 32]),
        op=ALU.is_ge,
    )
    sc = cp.tile([1, P], f32)
    nc.vector.tensor_tensor(out=sc[:], in0=gT[0:1, :], in1=eq[:], op=ALU.mult)
    smax = cp.tile([1, R], f32)
    nc.vector.reduce_max(
        out=smax[:], in_=sc[:].rearrange("p (r q) -> p r q", q=32),
        axis=mybir.AxisListType.X,
    )
    tstar = cp.tile([1, R], f32)
    nc.vector.tensor_scalar(
        out=tstar[:], in0=smax[:], scalar1=-1.0, scalar2=8191.0,
        op0=ALU.mult, op1=ALU.add,
    )
    lo = cp.tile([1, R], f32)
    nc.vector.tensor_scalar(
        out=lo[:], in0=tstar[:], scalar1=float(ATT_RANGE - 1), scalar2=0.0,
        op0=ALU.subtract, op1=ALU.max,
    )
    s0 = cp.tile([1, R], f32)
    nc.vector.tensor_scalar_min(s0[:], lo[:], float(T - W))
    s0i = cp.tile([1, R], i32)
    nc.vector.tensor_copy(s0i[:], s0[:])
    # spread carries the (p&31)+T*(p>>5) offset so the transpose output IS
    # the gather row index (minus nothing)
    sp0 = cp.tile([1, P], f32)
    nc.vector.tensor_tensor(
        out=sp0[:].rearrange("p (r q) -> p r q", q=32),
        in0=s0[:].unsqueeze(2).to_broadcast([1, R, 32]),
        in1=sm[0:1, C_SP : C_SP + P].rearrange("p (r q) -> p r q", q=32),
        op=ALU.add,
    )
    with tc.high_priority():
        spT = pt.tile([P, 1], f32, tag="tp")
        nc.tensor.transpose(out=spT[:, 0:1], in_=sp0[:], identity=ident[0:1, 0:1])
    soffs = cp.tile([P, 1], i32)
    nc.vector.tensor_copy(soffs[:], spT[:, 0:1])

    # ================= gathers ===========================================
    X = wp.tile([P, ENC_H], bf16)
    with tc.high_priority():
        encg = nc.gpsimd.indirect_dma_start(
            out=X[:],
            out_offset=None,
            in_=x["enc"][:],
            in_offset=bass.IndirectOffsetOnAxis(ap=soffs[:, 0:1], axis=0),
        )
    pawT = wp.tile([KW, P], bf16)
    svals = []
    for r in range(R):
        sv = nc.values_load(
            s0i[0:1, r : r + 1],
            engines=(mybir.EngineType.SP,),
            min_val=0,
            max_val=T - W,
            skip_runtime_bounds_check=True,
        )
        svals.append(sv)
        row = x["pa_pad"][r : r + 1, :]
        sl = row[0:1, bass.ds(sv, PAD + W + PAD + 1)]
        win = bass.AP(sl.tensor, sl.offset, [[1, KW], [1, W]])
        nc.sync.dma_start(out=pawT[0:KW, r * W : (r + 1) * W], in_=win)

    # ====== band mask + hi/lo + scatter offsets on Pool (slack work) ======
    # scheduling-only edges: keep the gather ahead of this group on Pool
    def _after_gather(ins_obj):
        tile.add_dep_helper(ins_obj.ins, encg.ins, sync=False, reason="gather first")
        return ins_obj

    hi = cp.tile([1, R], f32)
    _after_gather(nc.gpsimd.tensor_scalar_add(hi[:], tstar[:], float(ATT_RANGE - 1)))
    nc.gpsimd.tensor_tensor(
        out=hi[:], in0=hi[:], in1=sm[0:1, C_LM1 : C_LM1 + R], op=ALU.min
    )
    pos = cp.tile([1, P], f32)
    _after_gather(nc.gpsimd.tensor_tensor(
        out=pos[:], in0=sp0[:], in1=sm[0:1, C_JI : C_JI + P], op=ALU.add
    ))
    loB = cp.tile([1, P], f32)
    _after_gather(nc.gpsimd.tensor_copy(
        loB[:].rearrange("p (r q) -> p r q", q=32),
        lo[:].unsqueeze(2).to_broadcast([1, R, 32]),
    ))
    hiB = cp.tile([1, P], f32)
    _after_gather(nc.gpsimd.tensor_copy(
        hiB[:].rearrange("p (r q) -> p r q", q=32),
        hi[:].unsqueeze(2).to_broadcast([1, R, 32]),
    ))
    m1 = cp.tile([1, P], f32)
    nc.gpsimd.tensor_tensor(out=m1[:], in0=pos[:], in1=loB[:], op=ALU.is_ge)
    m2t = cp.tile([1, P], f32)
    nc.gpsimd.tensor_tensor(out=m2t[:], in0=pos[:], in1=hiB[:], op=ALU.is_le)
    nc.gpsimd.tensor_tensor(out=m1[:], in0=m1[:], in1=m2t[:], op=ALU.mult)
    masknegB = cp.tile([1, P], bf16)
    nc.gpsimd.tensor_scalar(
        out=masknegB[:], in0=m1[:], scalar1=1.0, scalar2=60.0,
        op0=ALU.subtract, op1=ALU.mult,
    )
    s0r4 = cp.tile([1, R], f32)
    _after_gather(nc.gpsimd.tensor_tensor(
        out=s0r4[:], in0=s0[:], in1=sm[0:1, C_R4 : C_R4 + R], op=ALU.add
    ))
    with tc.high_priority():
        s0T = pt.tile([R, 1], f32, tag="tp")
        nc.tensor.transpose(out=s0T[:, 0:1], in_=s0r4[:], identity=ident[0:1, 0:1])
    s0_4i = cp.tile([R, 1], i32)
    nc.gpsimd.tensor_copy(s0_4i[:], s0T[:, 0:1])

    # ================= prenet layers 2+3 on PE ============================
    PO = pq.tile([P, 4 * R], f32, tag="sm")
    for m2 in range(4):
        for q in range(8):
            nc.tensor.matmul(
                PO[:, m2 * R : (m2 + 1) * R],
                lhsT=w8[:, F_WP2 + q * DEC_H + m2 * P : F_WP2 + q * DEC_H + m2 * P + P],
                rhs=HTrB[:, q * R : (q + 1) * R],
                start=(q == 0), stop=(q == 7),
            )
    opTB = wp.tile([P, 4 * R], bf16)
    nc.scalar.activation(opTB[:], PO[:], AF.Relu, scale=1.0 / FS)
    PV = pq.tile([P, 2 * R], f32, tag="sm")
    PK = pq.tile([P, 2 * R], f32, tag="sm")
    for m in range(2):
        for q2 in range(4):
            nc.tensor.matmul(
                PV[:, m * R : (m + 1) * R],
                lhsT=w8[:, F_WDEC + q2 * ATT_H + m * P : F_WDEC + q2 * ATT_H + m * P + P],
                rhs=opTB[:, q2 * R : (q2 + 1) * R],
                start=(q2 == 0), stop=False,
            )
        nc.tensor.matmul(
            PV[:, m * R : (m + 1) * R],
            lhsT=ia[0:1, B_WSPD + m * P : B_WSPD + (m + 1) * P],
            rhs=ia[0:1, B_SPD : B_SPD + R], start=False, stop=True,
        )
        nc.tensor.matmul(
            PK[:, m * R : (m + 1) * R],
            lhsT=ia[0:SPK, B_WSPK + m * P : B_WSPK + (m + 1) * P],
            rhs=ia[0:SPK, B_SPK : B_SPK + R], start=True, stop=True,
        )

    # ================= prenet combine v (Pool) ============================
    v = wp.tile([P, 2 * R], f32)
    denk = cp.tile([P, 2 * R], f32)
    _after_gather(nc.gpsimd.tensor_scalar(
        out=denk[:], in0=PK[:], scalar1=0.0, scalar2=1.0,
        op0=ALU.abs_max, op1=ALU.add,
    ))
    nc.gpsimd.tensor_tensor(out=v[:], in0=PK[:], in1=denk[:], op=ALU.divide)
    # PV carries the x64 fp8-scaling of wdec/W_speed; undo it in the combine
    nc.gpsimd.scalar_tensor_tensor(
        out=v[:], in0=PV[:], scalar=1.0 / FS, in1=v[:],
        op0=ALU.mult, op1=ALU.add,
    )

    # ============== X transpose (PE) + copies (DVE/Act) ===================
    XT = wp.tile([P, ENC_H], bf16)
    for q in range(4):
        TQ = pt.tile([P, P], bf16, tag="tpb")
        nc.tensor.transpose(
            out=TQ[:], in_=X[:, q * P : (q + 1) * P], identity=identb[:]
        )
        if q % 2 == 0:
            nc.vector.tensor_copy(XT[:, q * P : (q + 1) * P], TQ[:])
        else:
            nc.scalar.copy(XT[:, q * P : (q + 1) * P], TQ[:])

    # ================= enc matmuls + conv =================================
    PS = pr.tile([P, 2 * P], f32, tag="mm")
    for m in range(2):
        for q in range(4):
            nc.tensor.matmul(
                PS[:, m * P : (m + 1) * P],
                lhsT=wenc[:, q * ATT_H + m * P : q * ATT_H + m * P + P],
                rhs=XT[:, q * P : (q + 1) * P],
                start=(q == 0), stop=(q == 3),
            )
    PC = pc.tile([P, 2 * P], f32, tag="pc")
    for m in range(2):
        nc.tensor.matmul(
            PC[:, m * P : (m + 1) * P],
            lhsT=ia[0:KW, B_CW + m * P : B_CW + (m + 1) * P],
            rhs=pawT[:],
            start=True, stop=True,
        )

    # ================= softsign + combine + tanh ==========================
    B = wp.tile([P, 2 * P], f32)
    nc.gpsimd.tensor_tensor(
        out=B[:].rearrange("p (m r j) -> p (m r) j", j=W, m=2),
        in0=PC[:].rearrange("p (m r j) -> p (m r) j", j=W, m=2),
        in1=v[:].unsqueeze(2).to_broadcast([P, 2 * R, W]),
        op=ALU.add,
    )
    den = wp.tile([P, 2 * P], bf16)
    nc.vector.tensor_scalar(
        out=den[:], in0=PS[:], scalar1=0.0, scalar2=1.0,
        op0=ALU.abs_max, op1=ALU.add,
    )
    e = wp.tile([P, 2 * P], bf16)
    nc.vector.tensor_tensor(out=e[:], in0=PS[:], in1=den[:], op=ALU.divide)
    nc.vector.tensor_tensor(out=e[:], in0=e[:], in1=B[:], op=ALU.add)
    th = wp.tile([P, 2 * P], bf16)
    nc.scalar.activation(th[:], e[:], AF.Tanh)

    # ============ project + mask-bias -> [1,128] free-major ===============
    PL = pq.tile([1, P], f32, tag="sm")
    for m in range(2):
        nc.tensor.matmul(
            PL[:], lhsT=ia[:, B_WPROJ + m : B_WPROJ + m + 1],
            rhs=th[:, m * P : (m + 1) * P],
            start=(m == 0), stop=False,
        )
    nc.tensor.matmul(
        PL[:], lhsT=ones1[0:1, 0:1], rhs=masknegB[:], start=False, stop=True
    )

    # ================= exp + row sums + divide + scatter ==================
    pex = cp.tile([1, P], f32)
    nc.scalar.activation(pex[:], PL[:], AF.Exp)
    den2 = cp.tile([1, R], f32)
    nc.vector.reduce_sum(
        out=den2[:], in_=pex[:].rearrange("p (r q) -> p r q", q=32),
        axis=mybir.AxisListType.X,
    )
    rden = cp.tile([1, R], f32)
    nc.vector.reciprocal(rden[:], den2[:])
    vals = cp.tile([1, P], f32)
    nc.vector.tensor_tensor(
        out=vals[:].rearrange("p (r q) -> p r q", q=32),
        in0=pex[:].rearrange("p (r q) -> p r q", q=32),
        in1=rden[:].unsqueeze(2).to_broadcast([1, R, 32]),
        op=ALU.mult,
    )
    d = nc.gpsimd.indirect_dma_start(
        out=x["out"][:],
        out_offset=bass.IndirectOffsetOnAxis(ap=s0_4i[:, 0:1], axis=1),
        in_=vals[:],
        in_offset=None,
    )
    tile.add_dep_helper(d.ins, zero_dma.ins, reason="scatter after zero")


def _split_sync_waits(nc, cap: int = 1):
    f = nc.m.functions[0]
    uid = [0]
    for blk in f.blocks:
        insts = blk.instructions
        out = []
        for inst in insts:
            si = inst.sync_info
            waits = list(si.on_wait) if (si is not None and si.on_wait) else []
            if len(waits) > cap:
                keep, excess = waits[:cap], waits[cap:]
                for k in range(0, len(excess), cap):
                    nop = mybir.InstEventSemaphore(
                        name=f"{inst.name}-ws{uid[0]}",
                        engine=inst.engine,
                        ins=[],
                        outs=[],
                        sync_info=mybir.SyncInfo(
                            on_wait=excess[k : k + cap], on_update=[]
                        ),
                    )
                    uid[0] += 1
                    out.append(nop)
                inst.sync_info = mybir.SyncInfo(
                    on_wait=keep, on_update=list(si.on_update or [])
                )
            out.append(inst)
        blk.instructions = out


def build_graph(reps: int = 1, split_waits: bool = True):
    from contextlib import ExitStack

    nc = bass.Bass()
    x = {}
    x["img"] = nc.declare_dram_parameter("img", [P, SMC], f32, isOutput=False)
    x["imga"] = nc.declare_dram_parameter("imga", [P, SMB], bf16, isOutput=False)
    x["w8"] = nc.declare_dram_parameter("w8", [P, SMF], fp8, isOutput=False)
    x["wp1b"] = nc.declare_dram_parameter("wp1b", [17, 2 * DEC_H], fp8, isOutput=False)
    x["wenc"] = nc.declare_dram_parameter("wenc", [P, 4 * ATT_H], bf16, isOutput=False)
    x["pa_pad"] = nc.declare_dram_parameter("pa_pad", [R, TP], bf16, isOutput=False)
    x["enc"] = nc.declare_dram_parameter("enc", [R * T, ENC_H], bf16, isOutput=False)
    x["out"] = nc.declare_dram_parameter("out", [R, T], f32, isOutput=True)

    with tile.TileContext(nc) as tc:
        for _ in range(reps):
            with ExitStack() as ctx:
                _emit_core(nc, tc, ctx, x)
    if split_waits:
        _split_sync_waits(nc)
    return nc


def _fold(w, q, p=P):
    n = w.shape[1]
    return np.ascontiguousarray(
        w.reshape(q, p, n).transpose(1, 0, 2).reshape(p, q * n), dtype=np.float32
    )


def host_prep(inputs: dict) -> list:
    inp = {k: np.asarray(v) for k, v in inputs.items()}
    pa = inp["prev_attention"].astype(np.float32)[:, :, 0]
    enc = inp["input_enc"].astype(np.float32)
    dec = inp["input_dec"].astype(np.float32)[:, 0, :]
    spk = inp["spkr_vec"].astype(np.float32)[:, 0, :]
    spd = inp["speed"].astype(np.float32)
    lens = inp["lengths_enc"].astype(np.float32)

    # these biases are structurally zero in this model; the kernel omits them
    for bname in ("bp2", "b_enc", "b_proj"):
        assert np.abs(np.asarray(inp[bname], np.float32)).max() == 0.0, bname

    import ml_dtypes

    bft = ml_dtypes.bfloat16
    f8t = ml_dtypes.float8_e4m3

    ia_base = np.zeros((P, SMB), np.float32)
    ia_base[0:SPK, B_WSPK : B_WSPK + ATT_H] = np.asarray(inp["W_spkr"], np.float32)
    ia_base[0:1, B_WSPD : B_WSPD + ATT_H] = FS * np.asarray(
        inp["W_speed"], np.float32
    ).reshape(1, ATT_H)
    ia_base[:, B_WPROJ : B_WPROJ + 2] = np.asarray(inp["W_proj"], np.float32).reshape(
        2, P
    ).T
    ia_base[0:KW, B_CW : B_CW + ATT_H] = np.asarray(inp["conv_w"], np.float32)[
        :, 0, :
    ].T

    w8_img = np.zeros((P, SMF), np.float32)
    wp1_full = FS * np.asarray(inp["Wp1"], np.float32)
    w8_img[:, F_WP1 : F_WP1 + 2 * DEC_H] = wp1_full[0:P, :]
    w8_img[:, F_WP2 : F_WP2 + 8 * DEC_H] = FS * _fold(
        np.asarray(inp["Wp2"], np.float32), 8
    )
    w8_img[:, F_WDEC : F_WDEC + 4 * ATT_H] = FS * _fold(
        np.asarray(inp["W_dec"], np.float32), 4
    )
    # bp1 rides as an extra constant-1 input channel on the wp1b block
    wp1b_img = np.zeros((17, 2 * DEC_H), np.float32)
    wp1b_img[0:16, :] = wp1_full[P : P + 16, :]
    wp1b_img[16, :] = FS * np.asarray(inp["bp1"], np.float32)

    wenc_img = _fold(np.asarray(inp["W_enc"], np.float32), 4).astype(bft)

    img_base = np.zeros((P, SMC), np.float32)
    img_base[0:1, C_JI : C_JI + P] = np.repeat(-T * np.arange(R), W).reshape(1, P)
    p_ar = np.arange(P)
    img_base[:, C_J32] = (p_ar & 31) + T * (p_ar >> 5)
    img_base[:, C_CBF] = 8191.0 - (p_ar & 31) * 128.0
    img_base[0:1, C_R4 : C_R4 + R] = (np.arange(R) * T).reshape(1, R)
    img_base[0:1, C_SP : C_SP + P] = ((p_ar & 31) + T * (p_ar >> 5)).reshape(1, P)

    in_maps = []
    for cix in range(NCORES):
        rows = slice(cix * R, (cix + 1) * R)
        pa_pad = np.zeros((R, TP), np.float32)
        pa_pad[:, PAD : PAD + T] = pa[rows]
        img = img_base.copy()
        img[:, C_PA : C_PA + P] = pa[rows].reshape(P, P)
        img[0:1, C_LM1 : C_LM1 + R] = (lens[rows] - 1.0).reshape(1, R)
        ia = ia_base.copy()
        ds_t = np.concatenate([dec[rows], spk[rows]], axis=1).T
        ia[:, B_DST0 : B_DST0 + R] = ds_t[0:P, :]
        ia[0:16, B_DST1 : B_DST1 + R] = ds_t[P : P + 16, :]
        ia[16, B_DST1 : B_DST1 + R] = 1.0
        ia[0:SPK, B_SPK : B_SPK + R] = spk[rows].T
        ia[0:1, B_SPD : B_SPD + R] = spd[rows].reshape(1, R)
        m = {
            "img": img,
            "imga": ia.astype(bft),
            "w8": w8_img.astype(f8t),
            "wp1b": wp1b_img.astype(f8t),
            "wenc": wenc_img,
            "pa_pad": pa_pad.astype(bft),
            "enc": np.ascontiguousarray(enc[rows].reshape(R * T, ENC_H)).astype(bft),
        }
        in_maps.append(m)
    return in_maps


_CACHED = {}


def kernel(**inputs) -> np.ndarray:
    from concourse.bass_utils import run_bass_kernel_spmd

    if "nc" not in _CACHED:
        _CACHED["nc"] = build_graph()
    nc = _CACHED["nc"]
    in_maps = host_prep(inputs)
    res = run_bass_kernel_spmd(nc, in_maps, core_ids=list(range(NCORES)))
    out = np.empty((N, T, 1), np.float32)
    for cix in range(NCORES):
        out[cix * R : (cix + 1) * R, :, 0] = res.results[cix]["out"]
    return out


# revision 54
# speedup vs baseline: 1.0588x; 1.0588x over previous
"""Trainium2 Bass kernel for nn_ARAttention, v5.

Structure (per core, R=4 batch rows; data-parallel over 8 cores):
- argmax of prev_attention via 2-level max (per-partition max8/maxidx, PE
  transpose, free-major reduce with tie-safe min-index trick)
- band window [s0, s0+31] gathered from enc via one 128-descriptor SWDGE
  indirect DMA; conv windows via 4 register-offset HWDGE DMAs
- e = softsign(W_enc x + b) + conv + v  (v = prenet terms, fp8 weights
  with x64 scaling; biases are all zero in this model — bp1 is folded into
  wp1b via a constant-1 input channel, the rest are assert-checked zero)
- tanh -> W_proj matmul with -60 band-mask bias rows -> exp -> per-row
  normalize -> one 4-descriptor SWDGE indirect scatter into the zeroed
  output.

Engine discipline: DVE runs ONLY the argmax->soffs critical chain plus the
post-PS softsign chain; band mask, v-combine, and B=PC+v run on Pool; PE
order is transposes -> prenet -> enc matmuls; Act does relus/tanh/exp and
big-image DMA issue. This keeps Tile's sem-coalescing from serializing the
critical chain behind slack work.
"""

import numpy as np

import concourse.bass as bass
import concourse.mybir as mybir

# -- walrus "too many sync waits" workaround (same as v1/v2) ----------------
import concourse.tile as tile
from concourse.vector_clock import VectorClock, ScopedClock


def _patched_drain_and_barrier(self, tick_clock, wait_clock):
    nc = self.nc
    gc = list(tick_clock.global_clock)
    for p, tick in enumerate(gc):
        if tick > 0:
            nop = nc.sync.nop(nofuse=True)
            partial = VectorClock([tick if i == p else 0 for i in range(len(gc))])
            wait_clock.add_sem_waits(nop.ins, ScopedClock({None: partial}))
    nc.sync.drain()
    nc.all_engine_barrier()
    assert self.sems is not None
    popped = nc._tile_sem_poison_stack.pop()
    assert popped is self._sem_poison
    nc.clear_and_free_semaphores(list(self.sems.allocated().values()))
    nc.all_engine_barrier()


tile.TileContext._drain_and_barrier = _patched_drain_and_barrier
# ---------------------------------------------------------------------------

from concourse.masks import make_identity

f32 = mybir.dt.float32
bf16 = mybir.dt.bfloat16
fp8 = mybir.dt.float8e4
i32 = mybir.dt.int32
u32 = mybir.dt.uint32
AF = mybir.ActivationFunctionType
ALU = mybir.AluOpType

N, T, ENC_H, ATT_H, DEC_H, OUTD, SPK = 32, 4096, 512, 256, 512, 80, 64
ATT_RANGE, KW = 10, 31
NCORES = 8
R = N // NCORES
W = 32
PAD = 15
TP = PAD + T + PAD + 2   # 4128
P = 128
FS = 64.0                # fp8 weight scaling

# f32 image columns
C_PA = 0          # [128,128] pa reshaped (r t)->(a b), b=128
C_LM1 = 128       # [1,4] row0: lengths-1
C_JI = 132       # [1,128] row0: -T*(c>>5)  (mask pos correction)
C_J32 = 260       # [128,1] per-partition: (p&31) + T*(p>>5)
C_CBF = 261       # [128,1] per-partition: 8191 - (p&31)*128
C_R4 = 262        # [1,4] row0: r*T
C_SP = 266        # [1,128] row0: (c&31) + T*(c>>5) (soffs spread add)
SMC = 394

# bf16 image columns
B_DST0 = 0        # [128,4] dec_in rows 0..127 (transposed)
B_DST1 = 4        # [17,4] dec_in rows 128..143 + ones row
B_SPK = 8         # [64,4]
B_SPD = 12        # [1,4]
B_WSPK = 16       # [64,256] (x64 NOT applied; accumulated with PK alone)
B_WSPD = 272      # [1,256] x64
B_WPROJ = 528     # [128,2]
B_CW = 530        # [31,256] conv weights transposed
SMB = 786

# fp8 weight image columns (all x64-scaled)
F_WP1 = 0         # [128,1024]
F_WP2 = 1024      # [128,4096]
F_WDEC = 5120     # [128,1024]
SMF = 6144


def _emit_core(nc, tc, ctx, x):
    cp = ctx.enter_context(tc.tile_pool(name="cp", bufs=1))
    wp = ctx.enter_context(tc.tile_pool(name="wp", bufs=1))
    pt = ctx.enter_context(tc.tile_pool(name="pt", bufs=2, space="PSUM"))
    pq = ctx.enter_context(tc.tile_pool(name="pq", bufs=2, space="PSUM"))
    pr = ctx.enter_context(tc.tile_pool(name="pr", bufs=1, space="PSUM"))
    pc = ctx.enter_context(tc.tile_pool(name="pc", bufs=1, space="PSUM"))

    ident = cp.tile([P, P], f32)
    make_identity(nc, ident[:])
    identb = cp.tile([P, P], bf16)
    nc.scalar.copy(identb[:], ident[:])
    ones1 = cp.tile([1, P], bf16)
    nc.gpsimd.memset(ones1[:], 1.0)

    # ---- static input DMAs (issue order ~ need order) --------------------
    sm = wp.tile([P, SMC], f32)
    with tc.high_priority():
        nc.sync.dma_start(sm[:], x["img"][:])
    ia = wp.tile([P, SMB], bf16)
    nc.scalar.dma_start(ia[:], x["imga"][:])
    w8 = wp.tile([P, SMF], fp8)
    nc.scalar.dma_start(w8[:], x["w8"][:])
    wp1b = wp.tile([17, 2 * DEC_H], fp8)
    nc.sync.dma_start(wp1b[:], x["wp1b"][:])
    wenc = wp.tile([P, 4 * ATT_H], bf16)
    nc.sync.dma_start(wenc[:], x["wenc"][:])

    pa128 = sm[:, C_PA : C_PA + P]

    # ================= prenet layer 1 on PE ===============================
    PH = pq.tile([P, 8 * R], f32, tag="sm")
    for m in range(8):
        nc.tensor.matmul(
            PH[:, m * R : (m + 1) * R],
            lhsT=w8[:, F_WP1 + m * P : F_WP1 + (m + 1) * P],
            rhs=ia[:, B_DST0 : B_DST0 + R], start=True, stop=False,
        )
        nc.tensor.matmul(
            PH[:, m * R : (m + 1) * R],
            lhsT=wp1b[0:17, m * P : (m + 1) * P],
            rhs=ia[0:17, B_DST1 : B_DST1 + R], start=False, stop=True,
        )
    HTrB = wp.tile([P, 8 * R], bf16)
    nc.scalar.activation(HTrB[:], PH[:], AF.Relu, scale=1.0 / FS)

    # ================= argmax level 1 (DVE) ===============================
    mx8 = cp.tile([P, 8], f32)
    nc.vector.max(out=mx8[:], in_=pa128)
    mi8 = cp.tile([P, 8], u32)
    nc.vector.max_index(out=mi8[:], in_max=mx8[:], in_values=pa128)
    # gidxp = (8191 - cb) - mi  (min-index tie-break via inverted score)
    gidxp = cp.tile([P, 1], f32)
    nc.vector.tensor_tensor(
        out=gidxp[:], in0=sm[:, C_CBF : C_CBF + 1], in1=mi8[:, 0:1], op=ALU.subtract
    )
    with tc.high_priority():
        vT = pt.tile([1, P], f32, tag="tp")
        nc.tensor.transpose(out=vT[:], in_=mx8[:, 0:1], identity=ident[:])
        gT = pt.tile([1, P], f32, tag="tp")
        nc.tensor.transpose(out=gT[:], in_=gidxp[:], identity=ident[:])

    # ================= argmax level 2 (free-major [1,128]) ================
    M = cp.tile([1, R], f32)
    nc.vector.reduce_max(
        out=M[:], in_=vT[0:1, :].rearrange("p (r q) -> p r q", q=32),
        axis=mybir.AxisListType.X,
    )
    eq = cp.tile([1, P], f32)
    nc.vector.tensor_tensor(
        out=eq[:].rearrange("p (r q) -> p r q", q=32),
        in0=vT[0:1, :].rearrange("p (r q) -> p r q", q=32),
        in1=M[:].to_broadcast([1, R, 32]),
        op=ALU.is_ge,
    )
    sc = cp.tile([1, P], f32)
    nc.vector.tensor_tensor(out=sc[:], in0=gT[0:1, :], in1=eq[:], op=ALU.mult)
    smax = cp.tile([1, R], f32)
    nc.vector.reduce_max(
        out=smax[:], in_=sc[:].rearrange("p (r q) -> p r q", q=32),
        axis=mybir.AxisListType.X,
    )
    tstar = cp.tile([1, R], f32)
    nc.vector.tensor_scalar(
        out=tstar[:], in0=smax[:], scalar1=-1.0, scalar2=8191.0,
        op0=ALU.mult, op1=ALU.add,
    )
    lo = cp.tile([1, R], f32)
    nc.vector.tensor_scalar(
        out=lo[:], in0=tstar[:], scalar1=float(ATT_RANGE - 1), scalar2=0.0,
        op0=ALU.subtract, op1=ALU.max,
    )
    s0 = cp.tile([1, R], f32)
    nc.vector.tensor_scalar_min(s0[:], lo[:], float(T - W))
    s0i = cp.tile([1, R], i32)
    nc.vector.tensor_copy(s0i[:], s0[:])
    # spread carries the (p&31)+T*(p>>5) offset so the transpose output IS
    # the gather row index (minus nothing)
    sp0 = cp.tile([1, P], f32)
    nc.vector.tensor_tensor(
        out=sp0[:].rearrange("p (r q) -> p r q", q=32),
        in0=s0[:].unsqueeze(2).to_broadcast([1, R, 32]),
        in1=sm[0:1, C_SP : C_SP + P].rearrange("p (r q) -> p r q", q=32),
        op=ALU.add,
    )
    with tc.high_priority():
        spT = pt.tile([P, 1], f32, tag="tp")
        nc.tensor.transpose(out=spT[:, 0:1], in_=sp0[:], identity=ident[0:1, 0:1])
    soffs = cp.tile([P, 1], i32)
    nc.vector.tensor_copy(soffs[:], spT[:, 0:1])

    # ================= gathers ===========================================
    X = wp.tile([P, ENC_H], bf16)
    with tc.high_priority():
        encg = nc.gpsimd.indirect_dma_start(
            out=X[:],
            out_offset=None,
            in_=x["enc"][:],
            in_offset=bass.IndirectOffsetOnAxis(ap=soffs[:, 0:1], axis=0),
        )
    pawT = wp.tile([KW, P], bf16)
    svals = []
    for r in range(R):
        sv = nc.values_load(
            s0i[0:1, r : r + 1],
            engines=(mybir.EngineType.SP, mybir.EngineType.Activation),
            min_val=0,
            max_val=T - W,
            skip_runtime_bounds_check=True,
        )
        svals.append(sv)
        row = x["pa_pad"][r : r + 1, :]
        sl = row[0:1, bass.ds(sv, PAD + W + PAD + 1)]
        win = bass.AP(sl.tensor, sl.offset, [[1, KW], [1, W]])
        eng = nc.sync if r % 2 == 0 else nc.scalar
        eng.dma_start(out=pawT[0:KW, r * W : (r + 1) * W], in_=win)

    # ====== band mask + hi/lo + scatter offsets on Pool (slack work) ======
    # scheduling-only edges: keep the gather ahead of this group on Pool
    def _after_gather(ins_obj):
        tile.add_dep_helper(ins_obj.ins, encg.ins, sync=False, reason="gather first")
        return ins_obj

    hi = cp.tile([1, R], f32)
    _after_gather(nc.gpsimd.tensor_scalar_add(hi[:], tstar[:], float(ATT_RANGE - 1)))
    nc.gpsimd.tensor_tensor(
        out=hi[:], in0=hi[:], in1=sm[0:1, C_LM1 : C_LM1 + R], op=ALU.min
    )
    pos = cp.tile([1, P], f32)
    _after_gather(nc.gpsimd.tensor_tensor(
        out=pos[:], in0=sp0[:], in1=sm[0:1, C_JI : C_JI + P], op=ALU.add
    ))
    loB = cp.tile([1, P], f32)
    _after_gather(nc.gpsimd.tensor_copy(
        loB[:].rearrange("p (r q) -> p r q", q=32),
        lo[:].unsqueeze(2).to_broadcast([1, R, 32]),
    ))
    hiB = cp.tile([1, P], f32)
    _after_gather(nc.gpsimd.tensor_copy(
        hiB[:].rearrange("p (r q) -> p r q", q=32),
        hi[:].unsqueeze(2).to_broadcast([1, R, 32]),
    ))
    m1 = cp.tile([1, P], f32)
    nc.gpsimd.tensor_tensor(out=m1[:], in0=pos[:], in1=loB[:], op=ALU.is_ge)
    m2t = cp.tile([1, P], f32)
    nc.gpsimd.tensor_tensor(out=m2t[:], in0=pos[:], in1=hiB[:], op=ALU.is_le)
    nc.gpsimd.tensor_tensor(out=m1[:], in0=m1[:], in1=m2t[:], op=ALU.mult)
    masknegB = cp.tile([1, P], bf16)
    nc.gpsimd.tensor_scalar(
        out=masknegB[:], in0=m1[:], scalar1=1.0, scalar2=60.0,
        op0=ALU.subtract, op1=ALU.mult,
    )


    # ================= prenet layers 2+3 on PE ============================
    PO = pq.tile([P, 4 * R], f32, tag="sm")
    for m2 in range(4):
        for q in range(8):
            nc.tensor.matmul(
                PO[:, m2 * R : (m2 + 1) * R],
                lhsT=w8[:, F_WP2 + q * DEC_H + m2 * P : F_WP2 + q * DEC_H + m2 * P + P],
                rhs=HTrB[:, q * R : (q + 1) * R],
                start=(q == 0), stop=(q == 7),
            )
    opTB = wp.tile([P, 4 * R], bf16)
    nc.scalar.activation(opTB[:], PO[:], AF.Relu, scale=1.0 / FS)
    PV = pq.tile([P, 2 * R], f32, tag="sm")
    PK = pq.tile([P, 2 * R], f32, tag="sm")
    for m in range(2):
        for q2 in range(4):
            nc.tensor.matmul(
                PV[:, m * R : (m + 1) * R],
                lhsT=w8[:, F_WDEC + q2 * ATT_H + m * P : F_WDEC + q2 * ATT_H + m * P + P],
                rhs=opTB[:, q2 * R : (q2 + 1) * R],
                start=(q2 == 0), stop=False,
            )
        nc.tensor.matmul(
            PV[:, m * R : (m + 1) * R],
            lhsT=ia[0:1, B_WSPD + m * P : B_WSPD + (m + 1) * P],
            rhs=ia[0:1, B_SPD : B_SPD + R], start=False, stop=True,
        )
        nc.tensor.matmul(
            PK[:, m * R : (m + 1) * R],
            lhsT=ia[0:SPK, B_WSPK + m * P : B_WSPK + (m + 1) * P],
            rhs=ia[0:SPK, B_SPK : B_SPK + R], start=True, stop=True,
        )

    # ================= prenet combine v (Pool) ============================
    v = wp.tile([P, 2 * R], f32)
    denk = cp.tile([P, 2 * R], f32)
    _after_gather(nc.gpsimd.tensor_scalar(
        out=denk[:], in0=PK[:], scalar1=0.0, scalar2=1.0,
        op0=ALU.abs_max, op1=ALU.add,
    ))
    nc.gpsimd.tensor_tensor(out=v[:], in0=PK[:], in1=denk[:], op=ALU.divide)
    # PV carries the x64 fp8-scaling of wdec/W_speed; undo it in the combine
    nc.gpsimd.scalar_tensor_tensor(
        out=v[:], in0=PV[:], scalar=1.0 / FS, in1=v[:],
        op0=ALU.mult, op1=ALU.add,
    )

    # ============== X transpose (PE) + copies (DVE) =======================
    XT = wp.tile([P, ENC_H], bf16)
    for q in range(4):
        TQ = pt.tile([P, P], bf16, tag="tpb")
        nc.tensor.transpose(
            out=TQ[:], in_=X[:, q * P : (q + 1) * P], identity=identb[:]
        )
        nc.vector.tensor_copy(XT[:, q * P : (q + 1) * P], TQ[:])

    # ================= enc matmuls + conv =================================
    PS = pr.tile([P, 2 * P], f32, tag="mm")
    for m in range(2):
        for q in range(4):
            nc.tensor.matmul(
                PS[:, m * P : (m + 1) * P],
                lhsT=wenc[:, q * ATT_H + m * P : q * ATT_H + m * P + P],
                rhs=XT[:, q * P : (q + 1) * P],
                start=(q == 0), stop=(q == 3),
            )
    PC = pc.tile([P, 2 * P], f32, tag="pc")
    for m in range(2):
        nc.tensor.matmul(
            PC[:, m * P : (m + 1) * P],
            lhsT=ia[0:KW, B_CW + m * P : B_CW + (m + 1) * P],
            rhs=pawT[:],
            start=True, stop=True,
        )

    # ================= softsign + combine + tanh ==========================
    B = wp.tile([P, 2 * P], f32)
    nc.gpsimd.tensor_tensor(
        out=B[:].rearrange("p (m r j) -> p (m r) j", j=W, m=2),
        in0=PC[:].rearrange("p (m r j) -> p (m r) j", j=W, m=2),
        in1=v[:].unsqueeze(2).to_broadcast([P, 2 * R, W]),
        op=ALU.add,
    )
    den = wp.tile([P, 2 * P], bf16)
    nc.vector.tensor_scalar(
        out=den[:], in0=PS[:], scalar1=0.0, scalar2=1.0,
        op0=ALU.abs_max, op1=ALU.add,
    )
    e = wp.tile([P, 2 * P], bf16)
    nc.vector.tensor_tensor(out=e[:], in0=PS[:], in1=den[:], op=ALU.divide)
    nc.vector.tensor_tensor(out=e[:], in0=e[:], in1=B[:], op=ALU.add)
    th = wp.tile([P, 2 * P], bf16)
    nc.scalar.activation(th[:], e[:], AF.Tanh)

    # ============ project + mask-bias -> [1,128] free-major ===============
    PL = pq.tile([1, P], f32, tag="sm")
    for m in range(2):
        nc.tensor.matmul(
            PL[:], lhsT=ia[:, B_WPROJ + m : B_WPROJ + m + 1],
            rhs=th[:, m * P : (m + 1) * P],
            start=(m == 0), stop=False,
        )
    nc.tensor.matmul(
        PL[:], lhsT=ones1[0:1, 0:1], rhs=masknegB[:], start=False, stop=True
    )

    # ========== exp + row sums + divide -> staging row [1,132] ============
    pex = cp.tile([1, P], f32)
    nc.scalar.activation(pex[:], PL[:], AF.Exp)
    den2 = cp.tile([1, R], f32)
    nc.vector.reduce_sum(
        out=den2[:], in_=pex[:].rearrange("p (r q) -> p r q", q=32),
        axis=mybir.AxisListType.X,
    )
    rden = cp.tile([1, R], f32)
    nc.vector.reciprocal(rden[:], den2[:])
    stg = cp.tile([1, P + R], f32)
    # s0 per row rides in the last 4 cols (host places the windows)
    nc.gpsimd.tensor_copy(stg[:, P : P + R], s0[:])
    nc.vector.tensor_tensor(
        out=stg[:, 0:P].rearrange("p (r q) -> p r q", q=32),
        in0=pex[:].rearrange("p (r q) -> p r q", q=32),
        in1=rden[:].unsqueeze(2).to_broadcast([1, R, 32]),
        op=ALU.mult,
    )
    nc.sync.dma_start(out=x["out"][:], in_=stg[:])


def _split_sync_waits(nc, cap: int = 1):
    f = nc.m.functions[0]
    uid = [0]
    for blk in f.blocks:
        insts = blk.instructions
        out = []
        for inst in insts:
            si = inst.sync_info
            waits = list(si.on_wait) if (si is not None and si.on_wait) else []
            if len(waits) > cap:
                keep, excess = waits[:cap], waits[cap:]
                for k in range(0, len(excess), cap):
                    nop = mybir.InstEventSemaphore(
                        name=f"{inst.name}-ws{uid[0]}",
                        engine=inst.engine,
                        ins=[],
                        outs=[],
                        sync_info=mybir.SyncInfo(
                            on_wait=excess[k : k + cap], on_update=[]
                        ),
                    )
                    uid[0] += 1
                    out.append(nop)
                inst.sync_info = mybir.SyncInfo(
                    on_wait=keep, on_update=list(si.on_update or [])
                )
            out.append(inst)
        blk.instructions = out


def build_graph(reps: int = 1, split_waits: bool = True):
    from contextlib import ExitStack

    nc = bass.Bass()
    x = {}
    x["img"] = nc.declare_dram_parameter("img", [P, SMC], f32, isOutput=False)
    x["imga"] = nc.declare_dram_parameter("imga", [P, SMB], bf16, isOutput=False)
    x["w8"] = nc.declare_dram_parameter("w8", [P, SMF], fp8, isOutput=False)
    x["wp1b"] = nc.declare_dram_parameter("wp1b", [17, 2 * DEC_H], fp8, isOutput=False)
    x["wenc"] = nc.declare_dram_parameter("wenc", [P, 4 * ATT_H], bf16, isOutput=False)
    x["pa_pad"] = nc.declare_dram_parameter("pa_pad", [R, TP], bf16, isOutput=False)
    x["enc"] = nc.declare_dram_parameter("enc", [R * T, ENC_H], bf16, isOutput=False)
    x["out"] = nc.declare_dram_parameter("out", [1, P + R], f32, isOutput=True)

    with tile.TileContext(nc) as tc:
        for _ in range(reps):
            with ExitStack() as ctx:
                _emit_core(nc, tc, ctx, x)
    if split_waits:
        _split_sync_waits(nc)
    return nc


def _fold(w, q, p=P):
    n = w.shape[1]
    return np.ascontiguousarray(
        w.reshape(q, p, n).transpose(1, 0, 2).reshape(p, q * n), dtype=np.float32
    )


def host_prep(inputs: dict) -> list:
    inp = {k: np.asarray(v) for k, v in inputs.items()}
    pa = inp["prev_attention"].astype(np.float32)[:, :, 0]
    enc = inp["input_enc"].astype(np.float32)
    dec = inp["input_dec"].astype(np.float32)[:, 0, :]
    spk = inp["spkr_vec"].astype(np.float32)[:, 0, :]
    spd = inp["speed"].astype(np.float32)
    lens = inp["lengths_enc"].astype(np.float32)

    # these biases are structurally zero in this model; the kernel omits them
    for bname in ("bp2", "b_enc", "b_proj"):
        assert np.abs(np.asarray(inp[bname], np.float32)).max() == 0.0, bname

    import ml_dtypes

    bft = ml_dtypes.bfloat16
    f8t = ml_dtypes.float8_e4m3

    ia_base = np.zeros((P, SMB), np.float32)
    ia_base[0:SPK, B_WSPK : B_WSPK + ATT_H] = np.asarray(inp["W_spkr"], np.float32)
    ia_base[0:1, B_WSPD : B_WSPD + ATT_H] = FS * np.asarray(
        inp["W_speed"], np.float32
    ).reshape(1, ATT_H)
    ia_base[:, B_WPROJ : B_WPROJ + 2] = np.asarray(inp["W_proj"], np.float32).reshape(
        2, P
    ).T
    ia_base[0:KW, B_CW : B_CW + ATT_H] = np.asarray(inp["conv_w"], np.float32)[
        :, 0, :
    ].T

    w8_img = np.zeros((P, SMF), np.float32)
    wp1_full = FS * np.asarray(inp["Wp1"], np.float32)
    w8_img[:, F_WP1 : F_WP1 + 2 * DEC_H] = wp1_full[0:P, :]
    w8_img[:, F_WP2 : F_WP2 + 8 * DEC_H] = FS * _fold(
        np.asarray(inp["Wp2"], np.float32), 8
    )
    w8_img[:, F_WDEC : F_WDEC + 4 * ATT_H] = FS * _fold(
        np.asarray(inp["W_dec"], np.float32), 4
    )
    # bp1 rides as an extra constant-1 input channel on the wp1b block
    wp1b_img = np.zeros((17, 2 * DEC_H), np.float32)
    wp1b_img[0:16, :] = wp1_full[P : P + 16, :]
    wp1b_img[16, :] = FS * np.asarray(inp["bp1"], np.float32)

    wenc_img = _fold(np.asarray(inp["W_enc"], np.float32), 4).astype(bft)

    img_base = np.zeros((P, SMC), np.float32)
    img_base[0:1, C_JI : C_JI + P] = np.repeat(-T * np.arange(R), W).reshape(1, P)
    p_ar = np.arange(P)
    img_base[:, C_J32] = (p_ar & 31) + T * (p_ar >> 5)
    img_base[:, C_CBF] = 8191.0 - (p_ar & 31) * 128.0
    img_base[0:1, C_R4 : C_R4 + R] = (np.arange(R) * T).reshape(1, R)
    img_base[0:1, C_SP : C_SP + P] = ((p_ar & 31) + T * (p_ar >> 5)).reshape(1, P)

    in_maps = []
    for cix in range(NCORES):
        rows = slice(cix * R, (cix + 1) * R)
        pa_pad = np.zeros((R, TP), np.float32)
        pa_pad[:, PAD : PAD + T] = pa[rows]
        img = img_base.copy()
        img[:, C_PA : C_PA + P] = pa[rows].reshape(P, P)
        img[0:1, C_LM1 : C_LM1 + R] = (lens[rows] - 1.0).reshape(1, R)
        ia = ia_base.copy()
        ds_t = np.concatenate([dec[rows], spk[rows]], axis=1).T
        ia[:, B_DST0 : B_DST0 + R] = ds_t[0:P, :]
        ia[0:16, B_DST1 : B_DST1 + R] = ds_t[P : P + 16, :]
        ia[16, B_DST1 : B_DST1 + R] = 1.0
        ia[0:SPK, B_SPK : B_SPK + R] = spk[rows].T
        ia[0:1, B_SPD : B_SPD + R] = spd[rows].reshape(1, R)
        m = {
            "img": img,
            "imga": ia.astype(bft),
            "w8": w8_img.astype(f8t),
            "wp1b": wp1b_img.astype(f8t),
            "wenc": wenc_img,
            "pa_pad": pa_pad.astype(bft),
            "enc": np.ascontiguousarray(enc[rows].reshape(R * T, ENC_H)).astype(bft),
        }
        in_maps.append(m)
    return in_maps


_CACHED = {}


def kernel(**inputs) -> np.ndarray:
    from concourse.bass_utils import run_bass_kernel_spmd

    if "nc" not in _CACHED:
        _CACHED["nc"] = build_graph()
    nc = _CACHED["nc"]
    in_maps = host_prep(inputs)
    res = run_bass_kernel_spmd(nc, in_maps, core_ids=list(range(NCORES)))
    out = np.zeros((N, T, 1), np.float32)
    for cix in range(NCORES):
        stg = np.asarray(res.results[cix]["out"]).reshape(P + R)
        for r in range(R):
            s0 = int(stg[P + r])
            out[cix * R + r, s0 : s0 + W, 0] = stg[r * W : (r + 1) * W]
    return out
